# revision 1
# baseline (speedup 1.0000x reference)
"""Distributed multi-head attention kernel for 8 TRN2 NeuronCores.

Problem: hidden[2,2048,1024] -> QKV proj (16 heads, hd=64) -> softmax
attention -> out proj. f32 I/O, bf16 tensor-engine compute; optionally
fp8e4 probs/V for a DoubleRow ctx contraction (CTX_FP8).

Sharding: sequence-parallel. Flattened rows [4096, 1024] split into 8
chunks of 512 rows; cores 0-3 own batch 0, cores 4-7 batch 1. Each core
projects K^T for its own chunk and AllGathers it within its 4-core
batch group (hidden under Q/V projections); V is projected redundantly
for the full batch on every core (cheaper than a second, serialized
AllGather), written directly into SBUF. Q^T is local. Each core then
runs full 16-head attention + output projection for its 512 rows;
outputs are disjoint row blocks concatenated on the host.

Engine budget: ScalarE does ONLY the 128 softmax exp tiles (one ACT
table load, no Reciprocal table thrash). All PSUM evictions (q/k bias
adds via tensor_scalar, V bias adds, out bias) run on the DVE. The
softmax denominators are stashed per head (bf16 row 64 of the psc
stash), gathered through a DRAM hop into a [8,512] tile, inverted by
ONE DVE Reciprocal per 4-pair batch (lanes parallel), broadcast to all
partitions with the baseline's all-ones stationary matmul, and applied
with a DVE multiply - the first batch overlaps the second half of
attention.
"""

import numpy as np
import ml_dtypes

B, S, D, H, HD = 2, 2048, 1024, 16, 64
N_CORES = 8
ROWS = (B * S) // N_CORES          # 512 query rows per core
GROUP = 4                          # cores per batch group
P = 128
KT = D // P                        # 8 contraction tiles over hidden dim
KEYT = S // P                      # 16 key tiles per batch
HA = HD + 1                        # head slot width in v_aug

CTX_FP8 = True                     # fp8e4 probs + V, DoubleRow ctx

_CACHE: dict = {}

bf16 = ml_dtypes.bfloat16
f8 = ml_dtypes.float8_e4m3


def _build_graph():
    import concourse.mybir as mybir
    import concourse.tile as tile
    from concourse import bacc
    from contextlib import ExitStack

    dt = mybir.dt
    F32, BF16 = dt.float32, dt.bfloat16
    PDT = dt.float8e4 if CTX_FP8 else BF16
    AF = mybir.ActivationFunctionType
    ALU = mybir.AluOpType
    DR = mybir.MatmulPerfMode.DoubleRow

    nc = bacc.Bacc("TRN2", target_bir_lowering=False, debug=False,
                   enable_asserts=False, num_devices=N_CORES)

    hT = nc.dram_tensor("hT", [D, ROWS], BF16, kind="ExternalInput").ap()
    hTf = nc.dram_tensor("hTf", [D, S], BF16, kind="ExternalInput").ap()
    wq = nc.dram_tensor("wq", [D, D], BF16, kind="ExternalInput").ap()
    wk = nc.dram_tensor("wk", [D, D], BF16, kind="ExternalInput").ap()
    wv = nc.dram_tensor("wv", [D, D], BF16, kind="ExternalInput").ap()
    wo = nc.dram_tensor("wo", [D, D], BF16, kind="ExternalInput").ap()
    bvb = nc.dram_tensor("bvb", [P, D], BF16, kind="ExternalInput").ap()
    bob = nc.dram_tensor("bob", [P, D], BF16, kind="ExternalInput").ap()
    bqk = nc.dram_tensor("bqk", [P, 2 * KT], F32, kind="ExternalInput").ap()
    sel = nc.dram_tensor("sel", [8, 8 * HD], BF16, kind="ExternalInput").ap()
    out = nc.dram_tensor("out", [ROWS, D], F32, kind="ExternalOutput").ap()

    with tile.TileContext(nc) as tc, ExitStack() as top:
        dram = top.enter_context(tc.tile_pool(name="dram", bufs=1, space="DRAM"))
        pers = top.enter_context(tc.tile_pool(name="pers", bufs=1))
        attn = top.enter_context(tc.tile_pool(name="attn", bufs=1))

        HB = D // 2
        kb0 = dram.tile([HB, ROWS], BF16)               # kT bounce, jt 0-3
        kb1 = dram.tile([HB, ROWS], BF16)               # kT bounce, jt 4-7
        KTg0 = dram.tile([GROUP * HB, ROWS], BF16)      # gathered kT, jt 0-3
        KTg1 = dram.tile([GROUP * HB, ROWS], BF16)      # gathered kT, jt 4-7
        dden = dram.tile([1, H * ROWS], BF16)           # denominator hop

        ones_full = pers.tile([P, P], BF16)
        nc.vector.memset(ones_full[:], 1.0)
        bqk_sb = pers.tile([P, 2 * KT], F32)
        nc.sync.dma_start(bqk_sb[:], bqk[:])
        bvb_sb = pers.tile([P, D], BF16)
        nc.sync.dma_start(bvb_sb[:], bvb[:])
        bob_sb = pers.tile([P, D], BF16)
        nc.sync.dma_start(bob_sb[:], bob[:])
        # per-head q^T slots, zero-padded on the other head's 64 partitions
        # so score matmuls can contract over the full 128 partitions (keeps
        # the PE in 128x128 mode -> no tiling-mode drains)
        qT_sb = pers.tile([P, H * ROWS], BF16)
        nc.vector.memset(qT_sb[:], 0.0)
        sel_sb = pers.tile([8, 8 * HD], BF16)
        nc.sync.dma_start(sel_sb[:], sel[:])

        kt_sb = attn.tile([P, 4 * KT * ROWS], BF16)     # gathered K^T
        v_aug = attn.tile([P, KEYT * H * HA], PDT)      # [V_h | 1] slots
        nc.gpsimd.memset(v_aug[:], 1.0)
        # pair-packed normalized ctx^T: head 2j on partitions 0-63 of pair
        # slot j, head 2j+1 on partitions 64-127 (odd heads arrive via a
        # cross-partition SBUF DMA from ctx_odd)
        ctx_pair = attn.tile([P, (H // 2) * ROWS], BF16)
        ctx_odd = attn.tile([HD, (H // 2) * ROWS], BF16)


        with ExitStack() as proj:
            wpool = proj.enter_context(tc.tile_pool(name="wpool", bufs=1))
            epool = proj.enter_context(tc.tile_pool(name="epool", bufs=3))
            ps_proj = proj.enter_context(
                tc.tile_pool(name="ps_proj", bufs=3, space="PSUM"))

            # input DMAs, most-urgent first; wk/hT split per k-tile pair so
            # the first k^T matmuls can start before the full tensors land
            wk_sb = wpool.tile([P, KT * D], BF16)
            hT_sb = wpool.tile([P, KT * ROWS], BF16)
            wq_sb = wpool.tile([P, KT * D], BF16)
            wv_sb = wpool.tile([P, KT * D], BF16)
            hTf_sb = wpool.tile([P, KT * S], BF16)
            for kk in range(0, KT, 2):
                nc.sync.dma_start(
                    hT_sb[:, kk * ROWS:(kk + 2) * ROWS]
                    .rearrange("p (k f) -> p k f", f=ROWS),
                    hT[kk * P:(kk + 2) * P, :]
                    .rearrange("(k p) f -> p k f", p=P))
                nc.sync.dma_start(
                    wk_sb[:, kk * D:(kk + 2) * D]
                    .rearrange("p (k f) -> p k f", f=D),
                    wk[kk * P:(kk + 2) * P, :]
                    .rearrange("(k p) f -> p k f", p=P))

            # k^T projection -> kb0/kb1 (bias bk folded into DVE eviction);
            # the AllGather is split in two so attention on head pairs 0-3
            # can start as soon as the first half lands
            def kproj(ms, kbt):
                for m in ms:
                    ps = ps_proj.tile([P, ROWS], F32, name="ps")
                    for k in range(KT):
                        nc.tensor.matmul(
                            ps[:],
                            wk_sb[:, k * D + m * P: k * D + (m + 1) * P],
                            hT_sb[:, k * ROWS:(k + 1) * ROWS],
                            start=(k == 0), stop=(k == KT - 1))
                    ev = epool.tile([P, ROWS], BF16, name="ev")
                    nc.vector.tensor_scalar(
                        ev[:], ps[:], bqk_sb[:, KT + m: KT + m + 1], None,
                        ALU.add)
                    nc.sync.dma_start(
                        kbt[(m % 4) * P:(m % 4 + 1) * P, :], ev[:])

            kproj(range(0, 4), kb0)
            kproj(range(4, 8), kb1)
            nc.gpsimd.collective_compute(
                "AllGather", mybir.AluOpType.bypass,
                replica_groups=[[0, 1, 2, 3], [4, 5, 6, 7]],
                ins=[kb0.opt()], outs=[KTg0.opt()])
            nc.gpsimd.collective_compute(
                "AllGather", mybir.AluOpType.bypass,
                replica_groups=[[0, 1, 2, 3], [4, 5, 6, 7]],
                ins=[kb1.opt()], outs=[KTg1.opt()])

            # remaining input DMAs (kept off the pre-collective DMA queue)
            for kk in range(0, KT, 2):
                nc.sync.dma_start(
                    wq_sb[:, kk * D:(kk + 2) * D]
                    .rearrange("p (k f) -> p k f", f=D),
                    wq[kk * P:(kk + 2) * P, :]
                    .rearrange("(k p) f -> p k f", p=P))
            for kk in range(0, KT, 2):
                nc.sync.dma_start(
                    wv_sb[:, kk * D:(kk + 2) * D]
                    .rearrange("p (k f) -> p k f", f=D),
                    wv[kk * P:(kk + 2) * P, :]
                    .rearrange("(k p) f -> p k f", p=P))
                for mh in (0, 1):
                    nc.sync.dma_start(
                        hTf_sb[:].rearrange("p (k f) -> p k f", f=S)
                        [:, kk:kk + 2, mh * (S // 2):(mh + 1) * (S // 2)],
                        hTf[kk * P:(kk + 2) * P,
                            mh * (S // 2):(mh + 1) * (S // 2)]
                        .rearrange("(k p) f -> p k f", p=P))

            # gathered K^T halves into SBUF (jt 0-3 after AG1, 4-7 after AG2)
            for half, KTgh in ((0, KTg0), (1, KTg1)):
                for r in range(GROUP):
                    nc.sync.dma_start(
                        kt_sb[:, (r * KT + half * 4) * ROWS:
                              (r * KT + half * 4 + 4) * ROWS]
                        .rearrange("p (t f) -> p t f", f=ROWS),
                        KTgh[r * HB:(r + 1) * HB, :]
                        .rearrange("(t p) f -> p t f", p=P))

            # q^T projection -> per-head zero-padded slots (DVE eviction)
            for m in range(KT):
                ps = ps_proj.tile([P, ROWS], F32, name="ps")
                for k in range(KT):
                    nc.tensor.matmul(
                        ps[:],
                        wq_sb[:, k * D + m * P: k * D + (m + 1) * P],
                        hT_sb[:, k * ROWS:(k + 1) * ROWS],
                        start=(k == 0), stop=(k == KT - 1))
                for hh in (0, 1):
                    h = 2 * m + hh
                    po = hh * HD
                    nc.vector.tensor_scalar(
                        qT_sb[po:po + HD, h * ROWS:(h + 1) * ROWS],
                        ps[po:po + HD, :],
                        bqk_sb[po:po + HD, m:m + 1], None,
                        ALU.add)

            # full-batch V projection: n=0 (heads 0-7) first - loop A
            # consumes it; n=1 follows, filling the PE window while the
            # first AllGather's barrier+transfer completes
            for n in (0, 1):
                for mk in range(KEYT):
                    ps = ps_proj.tile([P, 512], F32, name="ps")
                    for k in range(KT):
                        nc.tensor.matmul(
                            ps[:],
                            hTf_sb[:, k * S + mk * P: k * S + (mk + 1) * P],
                            wv_sb[:, k * D + n * 512: k * D + (n + 1) * 512],
                            start=(k == 0), stop=(k == KT - 1))
                    base = (mk * H + n * 8) * HA
                    nc.vector.tensor_add(
                        v_aug[:, base: base + 8 * HA]
                        .rearrange("p (h a) -> p h a", a=HA)[:, :, 0:HD],
                        ps[:].rearrange("p (h d) -> p h d", d=HD),
                        bvb_sb[:, n * 512:(n + 1) * 512]
                        .rearrange("p (h d) -> p h d", d=HD))

        v4 = v_aug[:].rearrange("p (t h a) -> p t h a", h=H, a=HA)

        with ExitStack() as att:
            late = att.enter_context(tc.tile_pool(name="late", bufs=1))
            wo_sb = late.tile([P, KT * D], BF16)
            for kk in range(0, KT, 4):
                nc.sync.dma_start(
                    wo_sb[:, kk * D:(kk + 4) * D]
                    .rearrange("p (k f) -> p k f", f=D),
                    wo[kk * P:(kk + 4) * P, :]
                    .rearrange("(k p) f -> p k f", p=P))

            probs = att.enter_context(tc.tile_pool(name="probs", bufs=12))
            ipool = att.enter_context(tc.tile_pool(name="ipool", bufs=3))
            # per-head unnormalized ctx stash: [V-rows 0..63 | denom row 64]
            stash = late.tile([HA, H * ROWS], BF16)
            rtn = late.tile([8, 2 * ROWS], BF16)
            rtr = late.tile([8, 2 * ROWS], F32)
            rtrb = late.tile([8, 2 * ROWS], BF16)

            norm_mm = []   # deferred (head, psb-producer) closures

            with ExitStack() as attp:
                ps_s = attp.enter_context(
                    tc.tile_pool(name="ps_s", bufs=2, space="PSUM"))
                ps_ctx = attp.enter_context(
                    tc.tile_pool(name="ps_ctx", bufs=2, space="PSUM"))
                ps_x = attp.enter_context(
                    tc.tile_pool(name="ps_x", bufs=1, space="PSUM"))

                U = KEYT // 2
                LAG_U = 4
                pend = {}
                psc = {}

                def emit_scores(p, u):
                    tiles = []
                    for hh in (0, 1):
                        tiles.append(ps_s.tile([P, 2 * ROWS], F32,
                                               name="ps_sc"))
                    for half in (0, 1):
                        t = 2 * u + half
                        r, m = t // (KEYT // GROUP), t % (KEYT // GROUP)
                        for hh in (0, 1):
                            h = 2 * p + hh
                            jt = h // 2
                            nc.tensor.matmul(
                                tiles[hh][:, half * ROWS:(half + 1) * ROWS],
                                kt_sb[:, (r * KT + jt) * ROWS + m * P:
                                      (r * KT + jt) * ROWS + (m + 1) * P],
                                qT_sb[:, h * ROWS:(h + 1) * ROWS],
                                start=True, stop=True)
                    dve_exp = (p * U + u) % 5 == 4 if p < 4 else (p * U + u) % 3 == 2
                    for hh in (0, 1):
                        pt = probs.tile([P, 2 * ROWS], PDT, name="pt")
                        if dve_exp and hh == 1:
                            # Schraudolph fast exp on the DVE: int32
                            # i = a*s + b approximates the f32 bit pattern
                            # of exp(s/8); bitcast + fp8 store
                            ti = ipool.tile([P, 2 * ROWS], dt.int32,
                                            name="ti")
                            nc.vector.tensor_scalar(
                                ti[:], tiles[hh][:],
                                12102203.16 / 8.0, 1064866805.0,
                                ALU.mult, ALU.add)
                            nc.vector.tensor_copy(
                                pt[:], ti[:].bitcast(F32))
                        else:
                            nc.scalar.activation(pt[:], tiles[hh][:],
                                                 AF.Exp, scale=0.125)
                        pend[(2 * p + hh, u)] = pt

                def emit_ctx(p, u):
                    for hh in (0, 1):
                        h = 2 * p + hh
                        if u == 0:
                            psc[h] = ps_ctx.tile([HA, ROWS], F32, name="ps_c")
                        pt = pend.pop((h, u))
                        if CTX_FP8:
                            nc.tensor.matmul(
                                psc[h][:],
                                v4[:, 2 * u:2 * u + 2, h, :],
                                pt[:].rearrange("p (t f) -> p t f", f=ROWS),
                                start=(u == 0), stop=(u == U - 1),
                                perf_mode=DR)
                        else:
                            for half in (0, 1):
                                t = 2 * u + half
                                off = (t * H + h) * HA
                                nc.tensor.matmul(
                                    psc[h][:], v_aug[:, off: off + HA],
                                    pt[:, half * ROWS:(half + 1) * ROWS],
                                    start=(t == 0), stop=(t == KEYT - 1))
                        if u == U - 1:
                            # stash unnormalized ctx + denominator row
                            ps_c = psc.pop(h)
                            nc.vector.tensor_copy(
                                stash[:, h * ROWS:(h + 1) * ROWS], ps_c[:])

                def flush_norm(heads):
                    # denominators -> DRAM hop -> partitions 0..7 -> one
                    # DVE Reciprocal (lanes parallel) -> bf16 -> selector
                    # matmul broadcast -> DVE normalize multiply
                    h0 = heads[0]
                    nh = len(heads)
                    f = h0 // 8
                    blk = slice(f * ROWS, (f + 1) * ROWS)
                    nc.sync.dma_start(
                        dden[0:1, h0 * ROWS:(h0 + nh) * ROWS],
                        stash[HD:HD + 1, h0 * ROWS:(h0 + nh) * ROWS])
                    nc.sync.dma_start(
                        rtn[0:nh, blk],
                        dden[0:1, h0 * ROWS:(h0 + nh) * ROWS]
                        .rearrange("p (h f) -> (p h) f", f=ROWS))
                    nc.vector.reciprocal(rtr[0:nh, blk], rtn[0:nh, blk])
                    nc.vector.tensor_copy(rtrb[0:nh, blk], rtr[0:nh, blk])
                    for h in heads:
                        hh = h - h0
                        psb = ps_x.tile([P, ROWS], F32, name="psb")
                        nc.tensor.matmul(
                            psb[0:HD, :], sel_sb[:, hh * HD:(hh + 1) * HD],
                            rtrb[0:nh, blk],
                            start=True, stop=True)
                        if h % 2 == 0:
                            dst = ctx_pair[0:HD, (h // 2) * ROWS:
                                           (h // 2 + 1) * ROWS]
                        else:
                            dst = ctx_odd[:, (h // 2) * ROWS:
                                          (h // 2 + 1) * ROWS]
                        nc.vector.tensor_mul(
                            dst, stash[0:HD, h * ROWS:(h + 1) * ROWS],
                            psb[0:HD, :])
                        if h % 2 == 1:
                            nc.sync.dma_start(
                                ctx_pair[HD:P, (h // 2) * ROWS:
                                         (h // 2 + 1) * ROWS],
                                ctx_odd[:, (h // 2) * ROWS:
                                        (h // 2 + 1) * ROWS])

                NP2 = 4
                for p0 in (0, 4):
                    for G in range(NP2 * U + LAG_U + 1):
                        if LAG_U <= G < NP2 * U + LAG_U:
                            emit_ctx(p0 + (G - LAG_U) // U, (G - LAG_U) % U)
                        if G < NP2 * U:
                            emit_scores(p0 + G // U, G % U)
                    flush_norm(list(range(2 * p0, 2 * p0 + 8)))

            with ExitStack() as outp_s:
                ps_o = outp_s.enter_context(
                    tc.tile_pool(name="ps_o", bufs=2, space="PSUM"))
                opool = outp_s.enter_context(tc.tile_pool(name="opool", bufs=3))
                for m in range(ROWS // P):
                    for n in range(2):
                        ps = ps_o.tile([P, 512], F32, name="ps_out")
                        for j in range(H // 2):
                            nc.tensor.matmul(
                                ps[:],
                                ctx_pair[:, j * ROWS + m * P:
                                         j * ROWS + (m + 1) * P],
                                wo_sb[:, j * D + n * 512: j * D + (n + 1) * 512],
                                start=(j == 0), stop=(j == H // 2 - 1))
                        ot = opool.tile([P, 512], F32, name="ot")
                        nc.vector.tensor_add(
                            ot[:], ps[:], bob_sb[:, n * 512:(n + 1) * 512])
                        nc.sync.dma_start(
                            out[m * P:(m + 1) * P, n * 512:(n + 1) * 512],
                            ot[:])

    nc.compile()
    return nc


def _prep_inputs(hidden_states, Wq, bq, Wk, bk, Wv, bv, Wo, bo):
    hs = np.asarray(hidden_states, np.float32).reshape(B * S, D)
    wq = np.asarray(Wq, np.float32).astype(bf16)
    wk = np.asarray(Wk, np.float32).astype(bf16)
    wv = np.asarray(Wv, np.float32).astype(bf16)
    wo = np.asarray(Wo, np.float32).astype(bf16)
    bvb = np.ascontiguousarray(np.broadcast_to(
        np.asarray(bv, np.float32)[None], (P, D))).astype(bf16)
    bob = np.ascontiguousarray(np.broadcast_to(
        np.asarray(bo, np.float32)[None], (P, D))).astype(bf16)
    sel = np.zeros((8, 8 * HD), np.float32)
    for hh in range(8):
        sel[hh, hh * HD:(hh + 1) * HD] = 1.0
    sel = sel.astype(bf16)
    bqk = np.ascontiguousarray(np.concatenate(
        [np.asarray(bq, np.float32).reshape(KT, P).T,
         np.asarray(bk, np.float32).reshape(KT, P).T], 1).astype(np.float32))
    hTf = [np.ascontiguousarray(hs[b * S:(b + 1) * S].T).astype(bf16)
           for b in range(B)]
    in_maps = []
    for c in range(N_CORES):
        hT = np.ascontiguousarray(
            hs[c * ROWS:(c + 1) * ROWS].T).astype(bf16)
        in_maps.append({"hT": hT, "hTf": hTf[c // GROUP], "wq": wq, "wk": wk,
                        "wv": wv, "wo": wo, "bvb": bvb, "bob": bob,
                        "bqk": bqk, "sel": sel})
    return in_maps


def _run(inputs, trace=False):
    from concourse import bass_utils
    if "nc" not in _CACHE:
        _CACHE["nc"] = _build_graph()
    nc = _CACHE["nc"]
    in_maps = _prep_inputs(**inputs)
    res = bass_utils.run_bass_kernel_spmd(
        nc, in_maps, core_ids=list(range(N_CORES)), trace=trace)
    full = np.concatenate([res.results[c]["out"] for c in range(N_CORES)],
                          axis=0).reshape(B, S, D).astype(np.float32)
    return full, res


def kernel(**inputs) -> np.ndarray:
    full, _ = _run(inputs, trace=False)
    return full

